# revision 24
# baseline (speedup 1.0000x reference)
"""Trainium2 Bass kernel for fused linear cross-attention + 1x1 conv + LayerNorm.

Computation (per batch element b, N=4096 tokens, D=512 channels, H=8 heads):
    kq = x2[b].T viewed as [H, 64, N]; v = x1[b].T viewed as [H, 64, N]
    key   = softmax(kq over N);  query = softmax(kq over head-channels)
    context  = key @ v.T     [H, 64, 64]
    attended = context.T @ query  -> agg [512, N]
    y = conv_w @ agg + conv_b    -> [N, 1024]
    out = LayerNorm(y) * ln_w + ln_b

Sharding: pure data-parallel over batch B=8 across the 8 NeuronCores (one
batch element per core, no collectives).

Kernel-level choices (v5):
  - softmax without max-subtraction (inputs are unit-normal; exp is safe) so
    key/query share one exp(x2) pass; fp16 matmul operands, fp32 PSUM.
  - channels are shipped pairwise head-interleaved (within each 128-channel
    chunk, local index = k*2 + head_parity).  The query-softmax reciprocal
    then broadcasts over a [*, 2]-packed innermost dim, which keeps every
    phase-1 DVE op eligible for the 2x 16-bit fast path; the per-head-pair
    context blocking (4 matmuls of F=130 per 128 tokens) is unchanged.
  - head sums for the query softmax via a packed tensor_tensor add tree
    (3 halvings + 1 strided reduce), all 2x-eligible.
  - x1 windows [x1 128-chunk | ones ones]: the ones columns give the
    key-softmax denominators from the context accumulation for free.
  - conv bias folded into the fused attended-conv weights MT (query softmax
    rows sum to exactly 1 per head => adding (b - mean b)/8 to every MT row
    reproduces the bias).  LayerNorm mean folded into MT too (rows centered;
    row sums computed analytically as A @ rowsum(cwT) via F=1 matmuls), so
    the conv emits y - mu directly and per-tile LN is variance-only.
  - the block-diagonal A is built from the context PSUM with a constant
    parity mask (kl%2 == vl%2) shipped from the host.
  - phase 1 runs on 512-token quad chunks, one big instruction per engine;
    PE work (context matmuls + transposes) runs one quad behind everything
    else in long back-to-back bursts to hold the tensor engine's high
    p-state clock.  PSUM evac of transposed queries splits scalar/gpsimd.
  - conv phase: single-tile PSUM accumulators, 4 deep; LN stats alternate
    between vector (bn_stats) and scalar (Square activation + accumulator)
    per tile; normalize alternates vector/scalar likewise; fp16 output DMA.
"""

import numpy as np

B, N, D = 8, 4096, 512
HEADS = 8
E2 = 2 * D  # 1024
NQ = 8          # quad chunks (512 tokens each)
NT = 32         # conv token tiles (128 tokens each)
LN_EPS = 1e-5
XW = 512 + 4 * 130  # 1032

_CACHE = {}


def _build(apply_ln_affine: bool):
    import concourse.bacc as bacc
    import concourse.mybir as mybir
    import concourse.tile as tile
    from concourse.masks import make_identity

    f32 = mybir.dt.float32
    f16 = mybir.dt.float16
    f8 = mybir.dt.float8e4
    AF = mybir.ActivationFunctionType
    ALU = mybir.AluOpType
    AX = mybir.AxisListType
    PM = mybir.MatmulPerfMode

    nc = bacc.Bacc("TRN2", target_bir_lowering=False, debug=False)

    # xmix: [x2 (0:512) | 4 x (x1 128-chunk + 2 ones) (512:1032)], channels
    # pairwise head-interleaved within each 128-chunk.
    xmixd = nc.dram_tensor("xmix", [N, XW], f16, kind="ExternalInput")
    cwTd = nc.dram_tensor("convT", [D, E2], f16, kind="ExternalInput")
    cbd = nc.dram_tensor("convb", [1, E2], f16, kind="ExternalInput")
    amaskd = nc.dram_tensor("amask", [128, 128], f16, kind="ExternalInput")
    if apply_ln_affine:
        lnwd = nc.dram_tensor("lnw", [1, E2], f32, kind="ExternalInput")
        lnbd = nc.dram_tensor("lnb", [1, E2], f32, kind="ExternalInput")
    outd = nc.dram_tensor("out", [N, E2], f16, kind="ExternalOutput")

    with tile.TileContext(nc) as tc:
        with (
            tc.tile_pool(name="consts", bufs=1) as consts,
            tc.tile_pool(name="resident", bufs=1) as res,
            tc.tile_pool(name="small", bufs=10) as small,
            tc.tile_pool(name="xstream", bufs=4) as xs,
            tc.tile_pool(name="estream", bufs=3) as es_pool,
            tc.tile_pool(name="qstream", bufs=3) as qs,
            tc.tile_pool(name="outs", bufs=4) as outs,
        ):
            ident = consts.tile([128, 128], f16, tag="ident", name="ident")
            make_identity(nc, ident[:])
            ident8 = consts.tile([128, 128], f8, tag="ident8", name="ident8")
            nc.vector.tensor_copy(out=ident8[:], in_=ident[:])
            # aux row 0 = ones (bias-fold matmul);  cb_ext row 0 = (b - mb)/8
            aux = consts.tile([128, 128], f16, tag="aux", name="aux")
            nc.gpsimd.memset(aux[:], 0.0)
            nc.gpsimd.memset(aux[0:1, :], 1.0)
            cb_ext = consts.tile([128, E2], f16, tag="cb_ext", name="cb_ext")
            nc.gpsimd.memset(cb_ext[:], 0.0)
            nc.gpsimd.dma_start(out=cb_ext[0:1, :], in_=cbd[:, :])
            # qcm and MT are stored as fp8e4 scaled by 256 each => y is
            # scaled by 65536; LayerNorm is scale-invariant so only eps
            # needs the matching 65536^2 scale.
            eps_t = consts.tile([128, 1], f32, tag="eps", name="eps")
            nc.gpsimd.memset(eps_t[:], LN_EPS * 65536.0 * 65536.0)
            amask = consts.tile([128, 128], f16, tag="amask", name="amask")
            nc.gpsimd.dma_start(out=amask[:], in_=amaskd[:, :])

            cwT = [consts.tile([128, E2], f16, tag=f"cwT{j}", name=f"cwT{j}")
                   for j in range(4)]
            for j in range(4):
                nc.gpsimd.dma_start(out=cwT[j][:], in_=cwTd[j * 128:(j + 1) * 128, :])

            if apply_ln_affine:
                import concourse.bass as bass
                lnw_b = consts.tile([128, E2], f32, tag="lnw", name="lnw")
                lnb_b = consts.tile([128, E2], f32, tag="lnb", name="lnb")
                for (dst, srcd) in ((lnw_b, lnwd), (lnb_b, lnbd)):
                    src = srcd[:, :]
                    bcast = bass.AP(
                        tensor=src.tensor, offset=src.offset,
                        ap=[[0, 128]] + list(src.ap)[1:],
                    )
                    nc.gpsimd.dma_start(out=dst[:], in_=bcast)

            qcm = res.tile([128, 4, N], f8, tag="qcm", name="qcm")
            trash = res.tile([128, E2], f16, tag="trash", name="trash")

            # ---- Phase 1: exp, query softmax, transposes, context accumulation
            with tc.tile_pool(name="ph1psum", bufs=1, space="PSUM") as c0pool, \
                 tc.tile_pool(name="qtpsum", bufs=2, space="PSUM") as qtp:
                c0 = [c0pool.tile([128, 130], f32, tag=f"c0_{p}", name=f"c0_{p}")
                      for p in range(4)]

                xms, Es, qts = {}, {}, {}

                def issue_dma(c):
                    if c >= NQ:
                        return
                    xm = xs.tile([128, 4, XW], f16, tag="xm", name="xm")
                    src = xmixd[c * 512:(c + 1) * 512, :]
                    nc.sync.dma_start(
                        out=xm[:],
                        in_=src.rearrange("(cc p) w -> p cc w", p=128),
                    )
                    xms[c] = xm

                issue_dma(0)
                issue_dma(1)

                for c in range(NQ + 1):
                    if c < NQ:
                        issue_dma(c + 2)
                        xm = xms[c]
                        E = es_pool.tile([128, 4, D], f16, tag="E", name="E")
                        nc.scalar.activation(E[:], xm[:, :, 0:D], AF.Exp)
                        Es[c] = E

                        # head sums: packed add tree over k (channels are
                        # [p, k, hl] pairwise interleaved)
                        ev = E[:].rearrange("p cc (pp k hl) -> p cc pp k hl",
                                            pp=4, hl=2)
                        with nc.allow_low_precision(reason="head sums ~1e2"):
                            t1 = small.tile([128, 4, 4, 32, 2], f16, tag="t1",
                                            name="t1")
                            nc.gpsimd.tensor_tensor(
                                out=t1[:], in0=ev[:, :, :, 0:32, :],
                                in1=ev[:, :, :, 32:64, :], op=ALU.add)
                            t2 = small.tile([128, 4, 4, 16, 2], f16, tag="t2",
                                            name="t2")
                            nc.vector.tensor_tensor(
                                out=t2[:], in0=t1[:, :, :, 0:16, :],
                                in1=t1[:, :, :, 16:32, :], op=ALU.add)
                            t3 = small.tile([128, 4, 4, 8, 2], f16, tag="t3",
                                            name="t3")
                            nc.vector.tensor_tensor(
                                out=t3[:], in0=t2[:, :, :, 0:8, :],
                                in1=t2[:, :, :, 8:16, :], op=ALU.add)
                            cs = small.tile([128, 4, 4, 2], f16, tag="cs",
                                            name="cs")
                            nc.vector.tensor_reduce(
                                cs[:],
                                t3[:].rearrange("p cc pp k hl -> p cc pp hl k"),
                                axis=AX.X, op=ALU.add)
                            R = small.tile([128, 4, 4, 2], f32, tag="R",
                                           name="R")
                            nc.vector.reciprocal(R[:], cs[:])
                            # fold the fp8 x256 scale into R
                            R2 = small.tile([128, 4, 4, 2], f16, tag="R2",
                                            name="R2")
                            nc.gpsimd.tensor_scalar_mul(out=R2[:], in0=R[:],
                                                        scalar1=256.0)

                        # q in fp8 (x256) so transposes and evac stay fp8
                        q = qs.tile([128, 4, D], f8, tag="q", name="q")
                        qv = q[:].rearrange("p cc (pp k hl) -> p cc pp k hl",
                                            pp=4, hl=2)
                        rv = R2[:].unsqueeze(3).broadcast_to((128, 4, 4, 64, 2))
                        nc.vector.tensor_tensor(out=qv[:, 0:2], in0=ev[:, 0:2],
                                                in1=rv[:, 0:2], op=ALU.mult)
                        nc.gpsimd.tensor_tensor(out=qv[:, 2:4], in0=ev[:, 2:4],
                                                in1=rv[:, 2:4], op=ALU.mult)
                        qts[c] = q

                    # PE: one full quad behind -> long back-to-back bursts
                    # that hold the tensor engine's high p-state.
                    if c >= 1:
                        d = c - 1
                        xm_d, E_d = xms[d], Es[d]
                        for cc in range(4):
                            for p in range(4):
                                win = xm_d[:, cc, 512 + 130 * p: 642 + 130 * p]
                                nc.tensor.matmul(
                                    c0[p][:, :],
                                    E_d[:, cc, p * 128:(p + 1) * 128], win,
                                    start=(d == 0 and cc == 0),
                                    stop=(d == NQ - 1 and cc == 3),
                                )
                        # fp8 transposes need output element step 2
                        qt = qtp.tile([128, 4, 512, 2], f8, tag="qt", name="qt")
                        for cc in range(4):
                            for j in range(4):
                                nc.tensor.transpose(
                                    qt[:, cc, j * 128:(j + 1) * 128, 0:1]
                                    .rearrange("p n one -> p (n one)"),
                                    qts[d][:, cc, j * 128:(j + 1) * 128],
                                    ident8[:],
                                )
                        qts[d] = None
                        # evac transposed q into resident qcm (channel-major)
                        dst = qcm[:, :, d * 512:(d + 1) * 512].rearrange(
                            "p j (cc n) -> p j cc n", cc=4)
                        src = qt[:, :, :, 0:1].rearrange(
                            "p cc (j n) one -> p j cc (n one)", j=4)
                        nc.scalar.copy(out=dst[:, 0:2], in_=src[:, 0:2])
                        nc.vector.tensor_copy(out=dst[:, 2:4], in_=src[:, 2:4])

                # ---- context normalization -> masked (pair-striped) A
                A = [res.tile([128, 128], f16, tag=f"A{p}", name=f"A{p}")
                     for p in range(4)]
                for p in range(4):
                    rec = small.tile([128, 1], f32, tag="rrec", name="rrec")
                    nc.vector.reciprocal(rec[:], c0[p][:, 128:129])
                    am = small.tile([128, 128], f16, tag="am", name="am")
                    nc.vector.tensor_tensor(out=am[:], in0=c0[p][:, 0:128],
                                            in1=amask[:], op=ALU.mult)
                    nc.vector.tensor_scalar_mul(out=A[p][:], in0=am[:],
                                                scalar1=rec[:, :])

            # per-row sums of cwT (for analytic MT row centering)
            cwsum = [consts.tile([128, 1], f16, tag=f"cws{j}", name=f"cws{j}")
                     for j in range(4)]
            with nc.allow_low_precision(reason="rowsum in f16 is plenty"):
                for j in range(4):
                    nc.vector.tensor_reduce(cwsum[j][:], cwT[j][:], axis=AX.X,
                                            op=ALU.add)

            # ---- Fuse attended+conv (+bias +mean-centering) into MT
            # MT stored as fp8 x256 in j-pairs for DoubleRow matmuls.
            AT = [res.tile([128, 128], f16, tag=f"AT{p}", name=f"AT{p}")
                  for p in range(4)]
            MT8 = [res.tile([128, 2, E2], f8, tag=f"MT8{pr}", name=f"MT8{pr}")
                   for pr in range(2)]
            with tc.tile_pool(name="atpsum", bufs=2, space="PSUM") as atp, \
                 tc.tile_pool(name="mpsum", bufs=2, space="PSUM") as mp, \
                 tc.tile_pool(name="rmpsum", bufs=2, space="PSUM") as rmp:
                for p in range(4):
                    atps = atp.tile([128, 128], f16, tag="atps", name="atps")
                    nc.tensor.transpose(atps[:], A[p][:], ident[:])
                    nc.scalar.copy(out=AT[p][:], in_=atps[:])
                for p in range(4):
                    mps = mp.tile([128, E2], f32, tag="mps", name="mps")
                    for e in range(2):
                        esl = slice(e * 512, (e + 1) * 512)
                        nc.tensor.matmul(mps[:, esl], AT[p][:], cwT[p][:, esl],
                                         start=True, stop=False)
                        nc.tensor.matmul(mps[:, esl], aux[:], cb_ext[:, esl],
                                         start=False, stop=True)
                    rmps = rmp.tile([128, 1], f32, tag="rmps", name="rmps")
                    nc.tensor.matmul(rmps[:, :], AT[p][:], cwsum[p][:, :])
                    rmneg = small.tile([128, 1], f32, tag="rmneg", name="rmneg")
                    nc.vector.tensor_scalar_mul(out=rmneg[:], in0=rmps[:, :],
                                                scalar1=-256.0 / E2)
                    nc.scalar.activation(MT8[p // 2][:, p % 2, :], mps[:],
                                         AF.Identity, scale=256.0,
                                         bias=rmneg[:, 0:1])

            # ---- conv (mean-centered, bias folded) + LayerNorm scale
            with tc.tile_pool(name="ypsum", bufs=4, space="PSUM") as yp:
                for t in range(NT):
                    y = yp.tile([128, E2], f32, tag="y", name="y")
                    tok = slice(t * 128, (t + 1) * 128)
                    for pr in range(2):
                        for e in range(2):
                            esl = slice(e * 512, (e + 1) * 512)
                            nc.tensor.matmul(
                                y[:, esl], qcm[:, 2 * pr:2 * pr + 2, tok],
                                MT8[pr][:, :, esl],
                                start=(pr == 0), stop=(pr == 1),
                                perf_mode=PM.DoubleRow,
                            )

                    sd = small.tile([128, 1], f32, tag="sd", name="sd")
                    rr = small.tile([128, 1], f32, tag="rr", name="rr")
                    ot = outs.tile([128, E2], f16, tag="ot", name="ot")
                    if t % 2 == 0:
                        # vector path: bn_stats halves + aggregate
                        stats = small.tile([128, 2, 6], f32, tag="stats",
                                           name="stats")
                        for e in range(2):
                            nc.vector.bn_stats(stats[:, e, :],
                                               y[:, e * 512:(e + 1) * 512])
                        mv = small.tile([128, 2], f32, tag="mv", name="mv")
                        nc.vector.bn_aggr(mv[:], stats[:])
                        nc.scalar.activation(sd[:], mv[:, 1:2], AF.Sqrt,
                                             bias=eps_t[:, 0:1])
                        nc.vector.reciprocal(rr[:], sd[:])
                        nc.vector.tensor_scalar_mul(out=ot[:], in0=y[:],
                                                    scalar1=rr[:, :])
                    else:
                        # scalar path: Square activation with accumulator.
                        # (y/32)^2 summed over 1024 elems = mean(y^2) exactly.
                        ssq = small.tile([128, 1], f32, tag="ssq", name="ssq")
                        nc.scalar.activation(trash[:], y[:], AF.Square,
                                             scale=1.0 / 32.0, accum_out=ssq[:])
                        nc.scalar.activation(sd[:], ssq[:], AF.Sqrt,
                                             bias=eps_t[:, 0:1])
                        nc.vector.reciprocal(rr[:], sd[:])
                        nc.scalar.activation(ot[:], y[:], AF.Identity,
                                             scale=rr[:, 0:1])
                    if apply_ln_affine:
                        nc.vector.tensor_tensor(out=ot[:], in0=ot[:],
                                                in1=lnw_b[:], op=ALU.mult)
                        nc.vector.tensor_tensor(out=ot[:], in0=ot[:],
                                                in1=lnb_b[:], op=ALU.add)
                    nc.sync.dma_start(out=outd[tok, :], in_=ot[:])
                    del y, ot

    nc.compile()
    return nc


def _get_nc(apply_ln_affine: bool):
    key = ("nc", apply_ln_affine)
    if key not in _CACHE:
        _CACHE[key] = _build(apply_ln_affine)
    return _CACHE[key]


def _perm():
    # pairwise head-interleave: chunk p holds heads {2p, 2p+1}; local
    # channel index = k*2 + (h - 2p).  perm[new] = old = h*64 + k.
    perm = np.empty(D, dtype=np.int64)
    for p in range(4):
        for k in range(64):
            for hl in range(2):
                perm[128 * p + 2 * k + hl] = (2 * p + hl) * 64 + k
    return perm


def kernel(x1, x2, conv_w, conv_b, ln_w, ln_b, _trace=False, _trace_kwargs=None):
    from concourse.bass_utils import run_bass_kernel_spmd

    x1 = np.asarray(x1, dtype=np.float32)
    x2 = np.ascontiguousarray(np.asarray(x2, dtype=np.float32))
    conv_w = np.asarray(conv_w, dtype=np.float32)
    conv_b = np.asarray(conv_b, dtype=np.float32)
    ln_w = np.asarray(ln_w, dtype=np.float32)
    ln_b = np.asarray(ln_b, dtype=np.float32)

    apply_affine = not (
        np.all(ln_w == 1.0) and np.all(ln_b == 0.0)
    )
    nc = _get_nc(apply_affine)

    perm = _perm()
    convT = np.ascontiguousarray(conv_w.T[perm, :]).astype(np.float16)
    cb8 = ((conv_b - conv_b.mean()) / 8.0).reshape(1, -1).astype(np.float16)
    kl = np.arange(128)
    amask = ((kl[:, None] % 2) == (kl[None, :] % 2)).astype(np.float16)
    in_maps = []
    for b in range(B):
        xmix = np.empty((N, XW), dtype=np.float16)
        xmix[:, 0:512] = x2[b].astype(np.float16)[:, perm]
        x1h = x1[b].astype(np.float16)[:, perm]
        for p in range(4):
            base = 512 + 130 * p
            xmix[:, base:base + 128] = x1h[:, 128 * p:128 * (p + 1)]
            xmix[:, base + 128:base + 130] = 1.0
        m = {
            "xmix": xmix,
            "convT": convT,
            "convb": cb8,
            "amask": amask,
        }
        if apply_affine:
            m["lnw"] = np.ascontiguousarray(ln_w.reshape(1, -1))
            m["lnb"] = np.ascontiguousarray(ln_b.reshape(1, -1))
        in_maps.append(m)

    kw = dict(_trace_kwargs or {})
    res = run_bass_kernel_spmd(nc, in_maps, list(range(B)), trace=_trace, **kw)
    out = np.stack([res.results[b]["out"] for b in range(B)], axis=0).astype(np.float32)
    if _trace:
        _CACHE["last_results"] = res
    return out


# revision 32
# speedup vs baseline: 1.0247x; 1.0247x over previous
"""Trainium2 Bass kernel for fused linear cross-attention + 1x1 conv + LayerNorm.

Computation (per batch element b, N=4096 tokens, D=512 channels, H=8 heads):
    kq = x2[b].T viewed as [H, 64, N]; v = x1[b].T viewed as [H, 64, N]
    key   = softmax(kq over N);  query = softmax(kq over head-channels)
    context  = key @ v.T     [H, 64, 64]
    attended = context.T @ query  -> agg [512, N]
    y = conv_w @ agg + conv_b    -> [N, 1024]
    out = LayerNorm(y) * ln_w + ln_b

Sharding: pure data-parallel over batch B=8 across the 8 NeuronCores (one
batch element per core, no collectives).

Kernel-level choices (v5):
  - softmax without max-subtraction (inputs are unit-normal; exp is safe) so
    key/query share one exp(x2) pass; fp16 matmul operands, fp32 PSUM.
  - channels are shipped pairwise head-interleaved (within each 128-channel
    chunk, local index = k*2 + head_parity).  The query-softmax reciprocal
    then broadcasts over a [*, 2]-packed innermost dim, which keeps every
    phase-1 DVE op eligible for the 2x 16-bit fast path; the per-head-pair
    context blocking (4 matmuls of F=130 per 128 tokens) is unchanged.
  - head sums for the query softmax via a packed tensor_tensor add tree
    (3 halvings + 1 strided reduce), all 2x-eligible.
  - x1 windows [x1 128-chunk | ones ones]: the ones columns give the
    key-softmax denominators from the context accumulation for free.
  - conv bias folded into the fused attended-conv weights MT (query softmax
    rows sum to exactly 1 per head => adding (b - mean b)/8 to every MT row
    reproduces the bias).  LayerNorm mean folded into MT too (rows centered;
    row sums computed analytically as A @ rowsum(cwT) via F=1 matmuls), so
    the conv emits y - mu directly and per-tile LN is variance-only.
  - the block-diagonal A is built from the context PSUM with a constant
    parity mask (kl%2 == vl%2) shipped from the host.
  - phase 1 runs on 512-token quad chunks, one big instruction per engine;
    PE work (context matmuls + transposes) runs one quad behind everything
    else in long back-to-back bursts to hold the tensor engine's high
    p-state clock.  PSUM evac of transposed queries splits scalar/gpsimd.
  - conv phase: single-tile PSUM accumulators, 4 deep; LN stats alternate
    between vector (bn_stats) and scalar (Square activation + accumulator)
    per tile; normalize alternates vector/scalar likewise; fp16 output DMA.
"""

import numpy as np

B, N, D = 8, 4096, 512
HEADS = 8
E2 = 2 * D  # 1024
NQ = 8          # quad chunks (512 tokens each)
NT = 32         # conv token tiles (128 tokens each)
LN_EPS = 1e-5
XW = 512 + 4 * 130  # 1032

_CACHE = {}


def _build(apply_ln_affine: bool):
    import concourse.bacc as bacc
    import concourse.mybir as mybir
    import concourse.tile as tile
    from concourse.masks import make_identity

    f32 = mybir.dt.float32
    f16 = mybir.dt.float16
    f8 = mybir.dt.float8e4
    AF = mybir.ActivationFunctionType
    ALU = mybir.AluOpType
    AX = mybir.AxisListType
    PM = mybir.MatmulPerfMode

    nc = bacc.Bacc("TRN2", target_bir_lowering=False, debug=False)

    # xmix: [x2 (0:512) | 4 x (x1 128-chunk + 2 ones) (512:1032)], channels
    # pairwise head-interleaved within each 128-chunk.
    xmixd = nc.dram_tensor("xmix", [N, XW], f16, kind="ExternalInput")
    cwTd = nc.dram_tensor("convT", [D, E2], f16, kind="ExternalInput")
    cbd = nc.dram_tensor("convb", [1, E2], f16, kind="ExternalInput")
    cwsd = nc.dram_tensor("cwsum", [D, 1], f16, kind="ExternalInput")
    amaskd = nc.dram_tensor("amask", [128, 128], f16, kind="ExternalInput")
    if apply_ln_affine:
        lnwd = nc.dram_tensor("lnw", [1, E2], f32, kind="ExternalInput")
        lnbd = nc.dram_tensor("lnb", [1, E2], f32, kind="ExternalInput")
    outd = nc.dram_tensor("out", [N, E2], f16, kind="ExternalOutput")

    with tile.TileContext(nc) as tc:
        with (
            tc.tile_pool(name="consts", bufs=1) as consts,
            tc.tile_pool(name="resident", bufs=1) as res,
            tc.tile_pool(name="small", bufs=10) as small,
            tc.tile_pool(name="xstream", bufs=4) as xs,
            tc.tile_pool(name="estream", bufs=3) as es_pool,
            tc.tile_pool(name="qstream", bufs=3) as qs,
            tc.tile_pool(name="outs", bufs=4) as outs,
        ):
            ident = consts.tile([128, 128], f16, tag="ident", name="ident")
            make_identity(nc, ident[:])
            # aux row 0 = ones (bias-fold matmul);  cb_ext row 0 = (b - mb)/8
            aux = consts.tile([128, 128], f16, tag="aux", name="aux")
            nc.gpsimd.memset(aux[:], 0.0)
            nc.gpsimd.memset(aux[0:1, :], 1.0)
            cb_ext = consts.tile([128, E2], f16, tag="cb_ext", name="cb_ext")
            nc.gpsimd.memset(cb_ext[:], 0.0)
            nc.gpsimd.dma_start(out=cb_ext[0:1, :], in_=cbd[:, :])
            # qcm and MT are stored as fp8e4 scaled by 256 each => y is
            # scaled by 65536; LayerNorm is scale-invariant so only eps
            # needs the matching 65536^2 scale.
            eps_t = consts.tile([128, 1], f32, tag="eps", name="eps")
            nc.gpsimd.memset(eps_t[:], LN_EPS * 65536.0 * 65536.0)
            amask = consts.tile([128, 128], f16, tag="amask", name="amask")
            nc.gpsimd.dma_start(out=amask[:], in_=amaskd[:, :])

            cwT = [consts.tile([128, E2], f16, tag=f"cwT{j}", name=f"cwT{j}")
                   for j in range(4)]
            cwsum = [consts.tile([128, 1], f16, tag=f"cws{j}", name=f"cws{j}")
                     for j in range(4)]
            for j in range(4):
                nc.gpsimd.dma_start(out=cwT[j][:], in_=cwTd[j * 128:(j + 1) * 128, :])
                nc.gpsimd.dma_start(out=cwsum[j][:], in_=cwsd[j * 128:(j + 1) * 128, :])

            if apply_ln_affine:
                import concourse.bass as bass
                lnw_b = consts.tile([128, E2], f32, tag="lnw", name="lnw")
                lnb_b = consts.tile([128, E2], f32, tag="lnb", name="lnb")
                for (dst, srcd) in ((lnw_b, lnwd), (lnb_b, lnbd)):
                    src = srcd[:, :]
                    bcast = bass.AP(
                        tensor=src.tensor, offset=src.offset,
                        ap=[[0, 128]] + list(src.ap)[1:],
                    )
                    nc.gpsimd.dma_start(out=dst[:], in_=bcast)

            qcm = res.tile([128, 4, N], f8, tag="qcm", name="qcm")
            trash = res.tile([128, E2], f16, tag="trash", name="trash")

            # ---- Phase 1: exp, query softmax, transposes, context accumulation
            with tc.tile_pool(name="ph1psum", bufs=1, space="PSUM") as c0pool, \
                 tc.tile_pool(name="qtpsum", bufs=2, space="PSUM") as qtp:
                c0 = [c0pool.tile([128, 130], f32, tag=f"c0_{p}", name=f"c0_{p}")
                      for p in range(4)]

                xms, Es, qts = {}, {}, {}

                def issue_dma(c):
                    if c >= NQ:
                        return
                    xm = xs.tile([128, 4, XW], f16, tag="xm", name="xm")
                    src = xmixd[c * 512:(c + 1) * 512, :]
                    nc.sync.dma_start(
                        out=xm[:],
                        in_=src.rearrange("(cc p) w -> p cc w", p=128),
                    )
                    xms[c] = xm

                issue_dma(0)
                issue_dma(1)

                for c in range(NQ + 1):
                    if c < NQ:
                        issue_dma(c + 2)
                        xm = xms[c]
                        E = es_pool.tile([128, 4, D], f16, tag="E", name="E")
                        nc.scalar.activation(E[:], xm[:, :, 0:D], AF.Exp)
                        Es[c] = E

                        # head sums: packed add tree over k (channels are
                        # [p, k, hl] pairwise interleaved)
                        ev = E[:].rearrange("p cc (pp k hl) -> p cc pp k hl",
                                            pp=4, hl=2)
                        with nc.allow_low_precision(reason="head sums ~1e2"):
                            t1 = small.tile([128, 4, 4, 32, 2], f16, tag="t1",
                                            name="t1")
                            nc.gpsimd.tensor_tensor(
                                out=t1[:], in0=ev[:, :, :, 0:32, :],
                                in1=ev[:, :, :, 32:64, :], op=ALU.add)
                            t2 = small.tile([128, 4, 4, 16, 2], f16, tag="t2",
                                            name="t2")
                            nc.vector.tensor_tensor(
                                out=t2[:], in0=t1[:, :, :, 0:16, :],
                                in1=t1[:, :, :, 16:32, :], op=ALU.add)
                            t3 = small.tile([128, 4, 4, 8, 2], f16, tag="t3",
                                            name="t3")
                            nc.vector.tensor_tensor(
                                out=t3[:], in0=t2[:, :, :, 0:8, :],
                                in1=t2[:, :, :, 8:16, :], op=ALU.add)
                            cs = small.tile([128, 4, 4, 2], f16, tag="cs",
                                            name="cs")
                            nc.vector.tensor_reduce(
                                cs[:],
                                t3[:].rearrange("p cc pp k hl -> p cc pp hl k"),
                                axis=AX.X, op=ALU.add)
                            R = small.tile([128, 4, 4, 2], f16, tag="R",
                                           name="R")
                            nc.vector.reciprocal(R[:], cs[:])

                        q = qs.tile([128, 4, D], f16, tag="q", name="q")
                        qv = q[:].rearrange("p cc (pp k hl) -> p cc pp k hl",
                                            pp=4, hl=2)
                        rv = R[:].unsqueeze(3).broadcast_to((128, 4, 4, 64, 2))
                        nc.vector.tensor_tensor(out=qv[:, 0:2], in0=ev[:, 0:2],
                                                in1=rv[:, 0:2], op=ALU.mult)
                        nc.gpsimd.tensor_tensor(out=qv[:, 2:4], in0=ev[:, 2:4],
                                                in1=rv[:, 2:4], op=ALU.mult)
                        qts[c] = q

                    # PE: one full quad behind -> long back-to-back bursts
                    # that hold the tensor engine's high p-state.
                    if c >= 1:
                        d = c - 1
                        xm_d, E_d = xms[d], Es[d]
                        for cc in range(4):
                            for p in range(4):
                                win = xm_d[:, cc, 512 + 130 * p: 642 + 130 * p]
                                nc.tensor.matmul(
                                    c0[p][:, :],
                                    E_d[:, cc, p * 128:(p + 1) * 128], win,
                                    start=(d == 0 and cc == 0),
                                    stop=(d == NQ - 1 and cc == 3),
                                )
                        qt = qtp.tile([128, 4, 512], f16, tag="qt", name="qt")
                        for cc in range(4):
                            for j in range(4):
                                nc.tensor.transpose(
                                    qt[:, cc, j * 128:(j + 1) * 128],
                                    qts[d][:, cc, j * 128:(j + 1) * 128],
                                    ident[:],
                                )
                        qts[d] = None
                        # evac transposed q into resident qcm (channel-major),
                        # casting to fp8 with the x256 scale
                        dst = qcm[:, :, d * 512:(d + 1) * 512].rearrange(
                            "p j (cc n) -> p j cc n", cc=4)
                        src = qt[:].rearrange("p cc (j n) -> p j cc n", j=4)
                        nc.scalar.activation(dst[:, 0:2], src[:, 0:2], AF.Copy,
                                             scale=256.0)
                        nc.vector.tensor_scalar_mul(out=dst[:, 2:4],
                                                    in0=src[:, 2:4],
                                                    scalar1=256.0)

                # ---- context normalization -> masked (pair-striped) A
                A = [res.tile([128, 128], f16, tag=f"A{p}", name=f"A{p}")
                     for p in range(4)]
                for p in range(4):
                    rec = small.tile([128, 1], f32, tag="rrec", name="rrec")
                    nc.vector.reciprocal(rec[:], c0[p][:, 128:129])
                    am = small.tile([128, 128], f16, tag="am", name="am")
                    nc.vector.tensor_tensor(out=am[:], in0=c0[p][:, 0:128],
                                            in1=amask[:], op=ALU.mult)
                    nc.vector.tensor_scalar_mul(out=A[p][:], in0=am[:],
                                                scalar1=rec[:, :])

            # ---- Fuse attended+conv (+bias +mean-centering) into MT
            # MT stored as fp8 x256 in j-pairs for DoubleRow matmuls.
            AT = [res.tile([128, 128], f16, tag=f"AT{p}", name=f"AT{p}")
                  for p in range(4)]
            MT8 = [res.tile([128, 2, E2], f8, tag=f"MT8{pr}", name=f"MT8{pr}")
                   for pr in range(2)]
            with tc.tile_pool(name="atpsum", bufs=2, space="PSUM") as atp, \
                 tc.tile_pool(name="mpsum", bufs=2, space="PSUM") as mp, \
                 tc.tile_pool(name="rmpsum", bufs=2, space="PSUM") as rmp:
                for p in range(4):
                    atps = atp.tile([128, 128], f16, tag="atps", name="atps")
                    nc.tensor.transpose(atps[:], A[p][:], ident[:])
                    nc.scalar.copy(out=AT[p][:], in_=atps[:])
                for p in range(4):
                    mps = mp.tile([128, E2], f32, tag="mps", name="mps")
                    for e in range(2):
                        esl = slice(e * 512, (e + 1) * 512)
                        nc.tensor.matmul(mps[:, esl], AT[p][:], cwT[p][:, esl],
                                         start=True, stop=False)
                        nc.tensor.matmul(mps[:, esl], aux[:], cb_ext[:, esl],
                                         start=False, stop=True)
                    rmps = rmp.tile([128, 1], f32, tag="rmps", name="rmps")
                    nc.tensor.matmul(rmps[:, :], AT[p][:], cwsum[p][:, :])
                    rmneg = small.tile([128, 1], f32, tag="rmneg", name="rmneg")
                    nc.vector.tensor_scalar_mul(out=rmneg[:], in0=rmps[:, :],
                                                scalar1=-256.0 / E2)
                    nc.scalar.activation(MT8[p // 2][:, p % 2, :], mps[:],
                                         AF.Identity, scale=256.0,
                                         bias=rmneg[:, 0:1])

            # ---- conv (mean-centered, bias folded) + LayerNorm scale
            with tc.tile_pool(name="ypsum", bufs=4, space="PSUM") as yp:
                for t in range(NT):
                    y = yp.tile([128, E2], f32, tag="y", name="y")
                    tok = slice(t * 128, (t + 1) * 128)
                    for pr in range(2):
                        for e in range(2):
                            esl = slice(e * 512, (e + 1) * 512)
                            nc.tensor.matmul(
                                y[:, esl], qcm[:, 2 * pr:2 * pr + 2, tok],
                                MT8[pr][:, :, esl],
                                start=(pr == 0), stop=(pr == 1),
                                perf_mode=PM.DoubleRow,
                            )

                    sd = small.tile([128, 1], f32, tag="sd", name="sd")
                    rr = small.tile([128, 1], f32, tag="rr", name="rr")
                    ot = outs.tile([128, E2], f16, tag="ot", name="ot")
                    if t % 2 == 0:
                        # vector path: bn_stats halves + aggregate
                        stats = small.tile([128, 2, 6], f32, tag="stats",
                                           name="stats")
                        for e in range(2):
                            nc.vector.bn_stats(stats[:, e, :],
                                               y[:, e * 512:(e + 1) * 512])
                        mv = small.tile([128, 2], f32, tag="mv", name="mv")
                        nc.vector.bn_aggr(mv[:], stats[:])
                        nc.scalar.activation(sd[:], mv[:, 1:2], AF.Sqrt,
                                             bias=eps_t[:, 0:1])
                        nc.vector.reciprocal(rr[:], sd[:])
                        nc.vector.tensor_scalar_mul(out=ot[:], in0=y[:],
                                                    scalar1=rr[:, :])
                    else:
                        # scalar path: Square activation with accumulator.
                        # (y/32)^2 summed over 1024 elems = mean(y^2) exactly.
                        ssq = small.tile([128, 1], f32, tag="ssq", name="ssq")
                        nc.scalar.activation(trash[:], y[:], AF.Square,
                                             scale=1.0 / 32.0, accum_out=ssq[:])
                        nc.scalar.activation(sd[:], ssq[:], AF.Sqrt,
                                             bias=eps_t[:, 0:1])
                        nc.vector.reciprocal(rr[:], sd[:])
                        nc.scalar.activation(ot[:], y[:], AF.Identity,
                                             scale=rr[:, 0:1])
                    if apply_ln_affine:
                        nc.vector.tensor_tensor(out=ot[:], in0=ot[:],
                                                in1=lnw_b[:], op=ALU.mult)
                        nc.vector.tensor_tensor(out=ot[:], in0=ot[:],
                                                in1=lnb_b[:], op=ALU.add)
                    nc.sync.dma_start(out=outd[tok, :], in_=ot[:])
                    del y, ot

    nc.compile()
    return nc


def _get_nc(apply_ln_affine: bool):
    key = ("nc", apply_ln_affine)
    if key not in _CACHE:
        _CACHE[key] = _build(apply_ln_affine)
    return _CACHE[key]


def _perm():
    # pairwise head-interleave: chunk p holds heads {2p, 2p+1}; local
    # channel index = k*2 + (h - 2p).  perm[new] = old = h*64 + k.
    perm = np.empty(D, dtype=np.int64)
    for p in range(4):
        for k in range(64):
            for hl in range(2):
                perm[128 * p + 2 * k + hl] = (2 * p + hl) * 64 + k
    return perm


def kernel(x1, x2, conv_w, conv_b, ln_w, ln_b, _trace=False, _trace_kwargs=None):
    from concourse.bass_utils import run_bass_kernel_spmd

    x1 = np.asarray(x1, dtype=np.float32)
    x2 = np.ascontiguousarray(np.asarray(x2, dtype=np.float32))
    conv_w = np.asarray(conv_w, dtype=np.float32)
    conv_b = np.asarray(conv_b, dtype=np.float32)
    ln_w = np.asarray(ln_w, dtype=np.float32)
    ln_b = np.asarray(ln_b, dtype=np.float32)

    apply_affine = not (
        np.all(ln_w == 1.0) and np.all(ln_b == 0.0)
    )
    nc = _get_nc(apply_affine)

    perm = _perm()
    convT = np.ascontiguousarray(conv_w.T[perm, :]).astype(np.float16)
    cwsum = convT.astype(np.float32).sum(axis=1, keepdims=True).astype(np.float16)
    cb8 = ((conv_b - conv_b.mean()) / 8.0).reshape(1, -1).astype(np.float16)
    kl = np.arange(128)
    amask = ((kl[:, None] % 2) == (kl[None, :] % 2)).astype(np.float16)
    in_maps = []
    for b in range(B):
        xmix = np.empty((N, XW), dtype=np.float16)
        xmix[:, 0:512] = x2[b].astype(np.float16)[:, perm]
        x1h = x1[b].astype(np.float16)[:, perm]
        for p in range(4):
            base = 512 + 130 * p
            xmix[:, base:base + 128] = x1h[:, 128 * p:128 * (p + 1)]
            xmix[:, base + 128:base + 130] = 1.0
        m = {
            "xmix": xmix,
            "convT": convT,
            "convb": cb8,
            "cwsum": cwsum,
            "amask": amask,
        }
        if apply_affine:
            m["lnw"] = np.ascontiguousarray(ln_w.reshape(1, -1))
            m["lnb"] = np.ascontiguousarray(ln_b.reshape(1, -1))
        in_maps.append(m)

    kw = dict(_trace_kwargs or {})
    res = run_bass_kernel_spmd(nc, in_maps, list(range(B)), trace=_trace, **kw)
    out = np.stack([res.results[b]["out"] for b in range(B)], axis=0).astype(np.float32)
    if _trace:
        _CACHE["last_results"] = res
    return out
